# revision 2
# baseline (speedup 1.0000x reference)
import sys
sys.path.insert(0, '/opt/trn_rl_repo')
"""Deformable-attention Bass kernel v2 (one batch image per core).

vs baseline:
  - q computed ONCE (natural layout, bf16 matmuls); transposed gather map
    derived via PE transposes (baseline ran the 512x512x4096 projection twice)
  - attention AV uses vT stationary with 64 ones-columns so the PE broadcasts
    the softmax denominator across partitions; one DVE divide normalizes and
    writes the transposed output layout directly (baseline: 512 matmuls + 512
    weight loads + 32 transposes + reciprocal/multiply)
  - bf16 weights/x halve DMA traffic
"""
import numpy as np
import ml_dtypes
import concourse.bass as bass
import concourse.tile as tile
from concourse import bacc, mybir

F32 = mybir.dt.float32
F32R = mybir.dt.float32r
BF16 = mybir.dt.bfloat16
I32 = mybir.dt.int32
AF = mybir.ActivationFunctionType
OP = mybir.AluOpType

DIM = 512; INNER = 512; H = W = 64; S = H * W
G = 8; D = 64; HEADS = 8; GH = GW = 16; J = GH * GW
SCALE = D ** -0.5
C15 = 64.0 / 15.0
MROWS = S + 2  # per-group gather-map rows incl front/back guard


def host_constants():
    j_of = (np.arange(2)[None, :, None] * 128 + np.arange(128)[:, None, None])
    j_of = np.broadcast_to(j_of, (128, 2, 8)).reshape(128, 16)  # [p, m=t*8+g]
    meshA = (j_of // GW) * C15 - 0.5
    meshB = (j_of % GW) * C15 - 0.5
    return meshA.astype(np.float32), meshB.astype(np.float32)


def prep_weights(w_q, w_off1, b_off1, w_off2, w_kv, w_out, b_out):
    bf = ml_dtypes.bfloat16
    w_q = np.asarray(w_q, np.float32); w_kv = np.asarray(w_kv, np.float32)
    w_out = np.asarray(w_out, np.float32)
    W2 = np.zeros((2, INNER, G), np.float32)
    for g in range(G):
        for k in range(2):
            W2[k, g * D:(g + 1) * D, g] = np.asarray(w_off2, np.float32)[k]
    return {
        "w_qT": np.ascontiguousarray(w_q.T).astype(bf),
        "w_kT": np.ascontiguousarray(w_kv[:INNER].T).astype(bf),
        "w_vT": np.ascontiguousarray(w_kv[INNER:].T).astype(bf),
        "w_oT": np.ascontiguousarray(w_out.T).astype(bf),
        "w1v": np.ascontiguousarray(np.tile(np.asarray(w_off1, np.float32), G))[:, None],
        "b1v": np.ascontiguousarray(np.tile(np.asarray(b_off1, np.float32), G))[:, None],
        "W2x": np.ascontiguousarray(W2[0]).astype(bf),
        "W2y": np.ascontiguousarray(W2[1]).astype(bf),
        "b_out": np.asarray(b_out, np.float32)[:, None],
    }


def build(debug_taps=(), stage=9):
    nc = bacc.Bacc("TRN2", target_bir_lowering=False)
    x_in = nc.dram_tensor("x", [DIM, S], BF16, kind="ExternalInput")
    w_qT = nc.dram_tensor("w_qT", [DIM, INNER], BF16, kind="ExternalInput")
    w_kT = nc.dram_tensor("w_kT", [INNER, INNER], BF16, kind="ExternalInput")
    w_vT = nc.dram_tensor("w_vT", [INNER, INNER], BF16, kind="ExternalInput")
    w_oT = nc.dram_tensor("w_oT", [INNER, DIM], BF16, kind="ExternalInput")
    w1v = nc.dram_tensor("w1v", [INNER, 1], F32, kind="ExternalInput")
    b1v = nc.dram_tensor("b1v", [INNER, 1], F32, kind="ExternalInput")
    W2x = nc.dram_tensor("W2x", [INNER, G], BF16, kind="ExternalInput")
    W2y = nc.dram_tensor("W2y", [INNER, G], BF16, kind="ExternalInput")
    b_out = nc.dram_tensor("b_out", [DIM, 1], F32, kind="ExternalInput")
    y_out = nc.dram_tensor("y", [DIM, S], F32, kind="ExternalOutput")
    taps = {}
    for t, shape, dt in [("q", [DIM, S], F32), ("qt", [G * MROWS, D], F32),
                         ("idx", [128, 32], I32), ("wb", [128, 64], F32),
                         ("kvf", [DIM, J], F32), ("k", [INNER, J], F32),
                         ("vT", [J, 1024], F32), ("outT", [INNER, S], F32)]:
        if t in debug_taps:
            taps[t] = nc.dram_tensor("tap_" + t, shape, dt, kind="ExternalOutput")

    meshA_np, meshB_np = host_constants()
    meshA_d = nc.inline_tensor(meshA_np, "meshA")
    meshB_d = nc.inline_tensor(meshB_np, "meshB")

    def _pipeline(tc):
        # ---------------- persistent pool ----------------
        P0 = tc.alloc_tile_pool(name="P0", bufs=1)
        ident = P0.tile([128, 128], F32)
        from concourse.masks import make_identity
        make_identity(nc, ident[:])
        ident_bf = P0.tile([128, 128], BF16)
        nc.vector.tensor_copy(ident_bf[:], ident[:])
        meshA = P0.tile([128, 16], F32)
        meshB = P0.tile([128, 16], F32)
        nc.sync.dma_start(meshA[:], meshA_d.ap())
        nc.sync.dma_start(meshB[:], meshB_d.ap())
        w1_sb = P0.tile([128, 4], F32)
        b1_sb = P0.tile([128, 4], F32)
        nc.sync.dma_start(w1_sb[:], w1v.ap().rearrange("(c p) one -> p (c one)", p=128))
        nc.sync.dma_start(b1_sb[:], b1v.ap().rearrange("(c p) one -> p (c one)", p=128))
        W2x_sb = P0.tile([128, 4, G], BF16)
        W2y_sb = P0.tile([128, 4, G], BF16)
        nc.sync.dma_start(W2x_sb[:], W2x.ap().rearrange("(c p) g -> p c g", p=128))
        nc.sync.dma_start(W2y_sb[:], W2y.ap().rearrange("(c p) g -> p c g", p=128))
        bout_sb = P0.tile([128, 4], F32)
        nc.sync.dma_start(bout_sb[:], b_out.ap().rearrange("(c p) one -> p (c one)", p=128))
        IDX = P0.tile([128, 32], I32)
        Wb = P0.tile([128, 64], F32)
        kvf = P0.tile([128, 4, J], BF16)
        k_sb = P0.tile([128, 4, J], BF16)
        vT_sb = P0.tile([128, 2, 8 * 128], BF16)
        nc.vector.memset(vT_sb[:], 1.0)
        wo_sb = P0.tile([128, 4, DIM], BF16)
        nc.scalar.dma_start(wo_sb[:], w_oT.ap().rearrange("(c p) n -> p c n", p=128))
        wk_sb = P0.tile([128, 4, INNER], BF16)
        wv_sb = P0.tile([128, 4, INNER], BF16)
        nc.sync.dma_start(wk_sb[:], w_kT.ap().rearrange("(c p) n -> p c n", p=128))
        nc.scalar.dma_start(wv_sb[:], w_vT.ap().rearrange("(c p) n -> p c n", p=128))

        # DRAM scratch: per-group transposed q map with zero guard rows
        drp = tc.alloc_tile_pool(name="dr", bufs=1, space="DRAM")
        qt_map = drp.tile([G * MROWS, D], BF16)
        zt = P0.tile([G, 2, D], BF16)
        nc.vector.memset(zt[:], 0.0)
        guard_dst = bass.AP(tensor=qt_map[:].tensor, offset=qt_map[:].offset,
                            ap=[[MROWS * D, G], [(MROWS - 1) * D, 2], [1, D]])
        nc.sync.dma_start(guard_dst, zt[:])

        # long-lived pools first (LIFO release discipline)
        outT_pool = tc.alloc_tile_pool(name="otp", bufs=1)
        outT_sb = outT_pool.tile([128, 4, S], BF16)
        q_pool = tc.alloc_tile_pool(name="qp", bufs=1)
        q_sb = q_pool.tile([128, 4, S], BF16)

        # ---------------- phase A: load x, natural q ----------------
        wq_pool = tc.alloc_tile_pool(name="wqp", bufs=1)
        wq_sb = wq_pool.tile([128, 4, INNER], BF16)
        nc.gpsimd.dma_start(wq_sb[:], w_qT.ap().rearrange("(c p) n -> p c n", p=128))
        x_pool = tc.alloc_tile_pool(name="xp", bufs=1)
        x_sb = x_pool.tile([128, 4, 4, 1024], BF16)  # [p, q4, c, 1024]
        xap = x_in.ap()
        for q4 in range(4):
            src = bass.AP(tensor=xap.tensor, offset=q4 * 1024,
                          ap=[[S, 128], [128 * S, 4], [1, 1024]])
            nc.gpsimd.dma_start(x_sb[:, q4, :, :], src)

        # ---------------- offsets from x (early IDX/Wb) ----------------
        offp = tc.alloc_tile_pool(name="offp", bufs=1)
        psOff = tc.alloc_tile_pool(name="psOff", bufs=2, space="PSUM")
        t_sb = offp.tile([128, 4, J], BF16)
        for ic in range(4):
            pqd = psOff.tile([128, J], F32, tag="pqd")
            for c in range(4):
                rhs = bass.AP(tensor=x_sb[:].tensor,
                              offset=x_sb[:].offset + c * 1024,
                              ap=[list(x_sb[:].ap[0]), [4096, 4], [256, 4], [4, 16]])
                nc.tensor.matmul(pqd[:], wq_sb[:, c, ic * 128:(ic + 1) * 128],
                                 rhs, start=(c == 0), stop=(c == 3))
            nc.scalar.activation(t_sb[:, ic, :], pqd[:], AF.Gelu,
                                 bias=b1_sb[:, ic:ic + 1], scale=w1_sb[:, ic:ic + 1])
        offx = offp.tile([128, 16], F32)
        offy = offp.tile([128, 16], F32)
        for jt in range(2):
            pxt = psOff.tile([128, G], F32, tag="pxt")
            pyt = psOff.tile([128, G], F32, tag="pyt")
            for c in range(4):
                nc.tensor.matmul(pxt[:], t_sb[:, c, jt * 128:(jt + 1) * 128],
                                 W2x_sb[:, c, :], start=(c == 0), stop=(c == 3))
            for c in range(4):
                nc.tensor.matmul(pyt[:], t_sb[:, c, jt * 128:(jt + 1) * 128],
                                 W2y_sb[:, c, :], start=(c == 0), stop=(c == 3))
            nc.scalar.activation(offx[:, jt * 8:(jt + 1) * 8], pxt[:], AF.Tanh)
            nc.scalar.activation(offy[:, jt * 8:(jt + 1) * 8], pyt[:], AF.Tanh)
        _fc = [0]
        def f16():
            _fc[0] += 1
            return offp.tile([128, 16], F32, name=f"f16_{_fc[0]}", tag=f"f16_{_fc[0]}")

        xs = f16()
        ys = f16()
        nc.vector.scalar_tensor_tensor(out=xs[:], in0=offx[:], scalar=4.0 * C15,
                                       in1=meshA[:], op0=OP.mult, op1=OP.add)
        nc.vector.scalar_tensor_tensor(out=ys[:], in0=offy[:], scalar=4.0 * C15,
                                       in1=meshB[:], op0=OP.mult, op1=OP.add)

        def floor_of(src):
            _fc[0] += 1
            ti = offp.tile([128, 16], I32, name=f"i16_{_fc[0]}", tag=f"i16_{_fc[0]}")
            nc.vector.tensor_copy(ti[:], src)
            tf = f16()
            nc.vector.tensor_copy(tf[:], ti[:])
            gt = f16()
            nc.vector.tensor_tensor(out=gt[:], in0=tf[:], in1=src, op=OP.is_gt)
            fl = f16()
            nc.vector.tensor_tensor(out=fl[:], in0=tf[:], in1=gt[:], op=OP.subtract)
            return fl

        x0f = floor_of(xs[:])
        y0f = floor_of(ys[:])

        def in_range(v, lo, hi):
            a = f16()
            b2 = f16()
            r = f16()
            nc.vector.tensor_scalar(out=a[:], in0=v, scalar1=float(lo), scalar2=None,
                                    op0=OP.is_ge)
            nc.vector.tensor_scalar(out=b2[:], in0=v, scalar1=float(hi), scalar2=None,
                                    op0=OP.is_le)
            nc.vector.tensor_tensor(out=r[:], in0=a[:], in1=b2[:], op=OP.mult)
            return r

        vx0 = in_range(x0f[:], 0, 63)
        vx1 = in_range(x0f[:], -1, 62)
        vy0 = in_range(y0f[:], 0, 63)
        vy1 = in_range(y0f[:], -1, 62)
        wx1 = f16()
        wy1 = f16()
        nc.vector.tensor_tensor(out=wx1[:], in0=xs[:], in1=x0f[:], op=OP.subtract)
        nc.vector.tensor_tensor(out=wy1[:], in0=ys[:], in1=y0f[:], op=OP.subtract)
        wx0m = f16()
        wx1m = f16()
        wy0m = f16()
        wy1m = f16()
        nc.vector.scalar_tensor_tensor(out=wx0m[:], in0=wx1[:], scalar=1.0,
                                       in1=vx0[:], op0=OP.subtract, op1=OP.mult)
        nc.vector.tensor_scalar_mul(wx0m[:], wx0m[:], -1.0)
        nc.vector.tensor_tensor(out=wx1m[:], in0=wx1[:], in1=vx1[:], op=OP.mult)
        nc.vector.scalar_tensor_tensor(out=wy0m[:], in0=wy1[:], scalar=1.0,
                                       in1=vy0[:], op0=OP.subtract, op1=OP.mult)
        nc.vector.tensor_scalar_mul(wy0m[:], wy0m[:], -1.0)
        nc.vector.tensor_tensor(out=wy1m[:], in0=wy1[:], in1=vy1[:], op=OP.mult)
        nc.vector.tensor_tensor(out=Wb[:, 0:16], in0=wy0m[:], in1=wx0m[:], op=OP.mult)
        nc.vector.tensor_tensor(out=Wb[:, 16:32], in0=wy0m[:], in1=wx1m[:], op=OP.mult)
        nc.vector.tensor_tensor(out=Wb[:, 32:48], in0=wy1m[:], in1=wx0m[:], op=OP.mult)
        nc.vector.tensor_tensor(out=Wb[:, 48:64], in0=wy1m[:], in1=wx1m[:], op=OP.mult)
        xm = f16()
        ym0 = f16()
        ym1 = f16()
        nc.vector.tensor_scalar(out=xm[:], in0=x0f[:], scalar1=1.0, scalar2=0.0,
                                op0=OP.add, op1=OP.max)
        nc.vector.tensor_scalar_min(xm[:], xm[:], 64.0)
        nc.vector.tensor_scalar(out=ym0[:], in0=y0f[:], scalar1=0.0, scalar2=63.0,
                                op0=OP.max, op1=OP.min)
        nc.vector.tensor_scalar(out=ym1[:], in0=y0f[:], scalar1=1.0, scalar2=0.0,
                                op0=OP.add, op1=OP.max)
        nc.vector.tensor_scalar_min(ym1[:], ym1[:], 63.0)
        IDXf = offp.tile([128, 32], F32)
        nc.vector.scalar_tensor_tensor(out=IDXf[:, 0:16], in0=ym0[:], scalar=64.0,
                                       in1=xm[:], op0=OP.mult, op1=OP.add)
        nc.vector.scalar_tensor_tensor(out=IDXf[:, 16:32], in0=ym1[:], scalar=64.0,
                                       in1=xm[:], op0=OP.mult, op1=OP.add)
        nc.vector.tensor_copy(IDX[:], IDXf[:])

        if "idx" in taps:
            nc.sync.dma_start(taps["idx"].ap(), IDX[:])
        if "wb" in taps:
            nc.sync.dma_start(taps["wb"].ap(), Wb[:])

        psOff.release()
        offp.release()

        # ------------- main loop: q -> qt transposes -> gathers per ic -------------
        psQ = tc.alloc_tile_pool(name="psQ", bufs=4, space="PSUM")
        psT = tc.alloc_tile_pool(name="psT", bufs=2, space="PSUM")
        psT2 = tc.alloc_tile_pool(name="psT2", bufs=2, space="PSUM")
        qtv_pool = tc.alloc_tile_pool(name="qtv", bufs=3)
        gpool = tc.alloc_tile_pool(name="gp", bufs=3)
        qt_flat = qt_map[:]
        for ic in range(4):
            for blk in range(8):
                pq = psQ.tile([128, 512], F32, tag="pq")
                for c in range(4):
                    nc.tensor.matmul(pq[:], wq_sb[:, c, ic * 128:(ic + 1) * 128],
                                     x_sb[:, blk // 2, c,
                                          (blk % 2) * 512:(blk % 2) * 512 + 512],
                                     start=(c == 0), stop=(c == 3))
                dst = q_sb[:, ic, blk * 512:(blk + 1) * 512]
                if blk % 2 == 0:
                    nc.scalar.activation(dst, pq[:], AF.Copy)
                else:
                    nc.vector.tensor_copy(dst, pq[:])
            for q16 in range(4):
                qtv = qtv_pool.tile([128, 2, 4, 2, D], BF16, tag="qtv")
                for sub in range(2):
                    q8 = q16 * 2 + sub
                    pt = psT.tile([128, 4, 128], BF16, tag="pt")
                    for i in range(4):
                        nc.tensor.transpose(
                            pt[:, i, :],
                            q_sb[:, ic, (q8 * 4 + i) * 128:(q8 * 4 + i + 1) * 128],
                            ident_bf[:])
                    if sub == 0:
                        nc.scalar.activation(qtv[:, sub, :, :, :], pt[:], AF.Copy)
                    else:
                        nc.vector.tensor_copy(qtv[:, sub, :, :, :], pt[:])
                for gh in range(2):
                    dst = bass.AP(tensor=qt_map[:].tensor,
                                  offset=qt_map[:].offset
                                  + ((2 * ic + gh) * MROWS + 1 + q16 * 1024) * D,
                                  ap=[[D, 128], [128 * D, 8], [1, D]])
                    srcap = bass.AP(tensor=qtv[:].tensor,
                                    offset=qtv[:].offset + gh * D,
                                    ap=[list(qtv[:].ap[0]), [128, 8], [1, D]])
                    nc.sync.dma_start(dst, srcap)
            for gsub in range(2):
                g = 2 * ic + gsub
                Gt = gpool.tile([128, 512], BF16, tag="G")
                for yy in range(2):
                    for t in range(2):
                        col = yy * 16 + t * 8 + g
                        nc.gpsimd.indirect_dma_start(
                            out=Gt[:, (yy * 2 + t) * 128:(yy * 2 + t + 1) * 128],
                            out_offset=None, in_=qt_flat,
                            in_offset=bass.IndirectOffsetOnAxis(
                                ap=IDX[:, col:col + 1], axis=0),
                            element_offset=g * MROWS * D)
                for t in range(2):
                    acc = gpool.tile([128, D], F32, tag="acc")
                    m = t * 8 + g
                    nc.vector.tensor_scalar(out=acc[:], in0=Gt[:, t * 128:t * 128 + 64],
                                            scalar1=Wb[:, m:m + 1], scalar2=None,
                                            op0=OP.mult)
                    for yy, xx in ((0, 1), (1, 0), (1, 1)):
                        blk2 = (yy * 2 + t) * 128 + xx * 64
                        wcol = (2 * yy + xx) * 16 + m
                        nc.vector.scalar_tensor_tensor(
                            out=acc[:], in0=Gt[:, blk2:blk2 + 64],
                            scalar=Wb[:, wcol:wcol + 1], in1=acc[:],
                            op0=OP.mult, op1=OP.add)
                    pt2 = psT2.tile([64, 128], F32, tag="pt2")
                    nc.tensor.transpose(pt2[:], acc[:], ident[:])
                    nc.vector.tensor_copy(
                        kvf[(g % 2) * 64:(g % 2) * 64 + 64, g // 2,
                            t * 128:(t + 1) * 128],
                        pt2[:])
        if "q" in taps:
            dbgq = tc.alloc_tile_pool(name="dbgq", bufs=1)
            for ic in range(4):
                for blk in range(8):
                    ev = dbgq.tile([128, 512], F32, tag="qdbg")
                    nc.vector.tensor_copy(ev[:], q_sb[:, ic, blk * 512:(blk + 1) * 512])
                    nc.sync.dma_start(
                        taps["q"].ap()[ic * 128:(ic + 1) * 128,
                                       blk * 512:(blk + 1) * 512], ev[:])
            dbgq.release()
        if "idx" in taps:
            nc.sync.dma_start(taps["idx"].ap(), IDX[:])
        if "wb" in taps:
            nc.sync.dma_start(taps["wb"].ap(), Wb[:])
        if "kvf" in taps:
            dbgk = tc.alloc_tile_pool(name="dbgk", bufs=1)
            for c in range(4):
                kf = dbgk.tile([128, J], F32, tag="kvff")
                nc.vector.tensor_copy(kf[:], kvf[:, c, :])
                nc.sync.dma_start(taps["kvf"].ap()[c * 128:(c + 1) * 128, :], kf[:])
            dbgk.release()
        gpool.release(); qtv_pool.release()
        psT2.release(); psT.release(); psQ.release()
        x_pool.release(); wq_pool.release()
        if stage < 3:
            q_pool.release(); outT_pool.release(); drp.release(); P0.release()
            return

        # ---------------- k and vT ----------------
        psKV = tc.alloc_tile_pool(name="psKV", bufs=2, space="PSUM")
        for oc in range(4):
            pk = psKV.tile([128, J], F32, tag="pk")
            for c in range(4):
                nc.tensor.matmul(pk[:], wk_sb[:, c, oc * 128:(oc + 1) * 128],
                                 kvf[:, c, :], start=(c == 0), stop=(c == 3))
            if oc % 2 == 0:
                nc.scalar.activation(k_sb[:, oc, :], pk[:], AF.Copy)
            else:
                nc.vector.tensor_copy(k_sb[:, oc, :], pk[:])
        for jt in range(2):
            pv = psKV.tile([128, INNER], F32, tag="pv")
            for c in range(4):
                nc.tensor.matmul(pv[:], kvf[:, c, jt * 128:(jt + 1) * 128],
                                 wv_sb[:, c, :], start=(c == 0), stop=(c == 3))
            vdst = bass.AP(tensor=vT_sb[:].tensor,
                           offset=vT_sb[:].offset + jt * 1024,
                           ap=[list(vT_sb[:].ap[0]), [128, 8], [1, 64]])
            nc.vector.tensor_copy(vdst, pv[:].rearrange("p (h d) -> p h d", h=8))
        if "k" in taps or "vT" in taps:
            dbgkv = tc.alloc_tile_pool(name="dbgkv", bufs=1)
            if "k" in taps:
                for c in range(4):
                    kf2 = dbgkv.tile([128, J], F32, tag="ktap")
                    nc.vector.tensor_copy(kf2[:], k_sb[:, c, :])
                    nc.sync.dma_start(taps["k"].ap()[c * 128:(c + 1) * 128, :], kf2[:])
            if "vT" in taps:
                for jt in range(2):
                    vf = dbgkv.tile([128, 1024], F32, tag="vtap")
                    nc.vector.tensor_copy(vf[:], vT_sb[:, jt, :])
                    nc.sync.dma_start(taps["vT"].ap()[jt * 128:(jt + 1) * 128, :], vf[:])
            dbgkv.release()
        psKV.release()
        if stage < 4:
            q_pool.release(); outT_pool.release(); drp.release(); P0.release()
            return

        # ---------------- attention ----------------
        ep = tc.alloc_tile_pool(name="ep", bufs=2)
        zpool = tc.alloc_tile_pool(name="zp", bufs=3)
        psS = tc.alloc_tile_pool(name="psS", bufs=2, space="PSUM")
        psAV = tc.alloc_tile_pool(name="psAV", bufs=1, space="PSUM")
        for hp in range(4):
            E = ep.tile([128, 2, 2, S], BF16, tag="E")  # [j, hh, jt, s]
            for jt in range(2):
                for blk in range(8):
                    ps2 = psS.tile([128, 2, 512], F32, tag="ps2", name="ps2")
                    nc.tensor.matmul(
                        ps2[:, 0, :],
                        k_sb[0:64, hp, jt * 128:(jt + 1) * 128],
                        q_sb[0:64, hp, blk * 512:(blk + 1) * 512],
                        start=True, stop=True)
                    nc.tensor.matmul(
                        ps2[:, 1, :],
                        k_sb[64:128, hp, jt * 128:(jt + 1) * 128],
                        q_sb[64:128, hp, blk * 512:(blk + 1) * 512],
                        start=True, stop=True)
                    eout = bass.AP(
                        tensor=E[:].tensor,
                        offset=E[:].offset + jt * S + blk * 512,
                        ap=[list(E[:].ap[0]), [2 * S, 2], [1, 512]])
                    nc.scalar.activation(eout, ps2[:], AF.Exp, scale=SCALE)
            for sq in range(4):
                pavA = psAV.tile([128, 1024], F32, tag="pavA")
                pavB = psAV.tile([128, 1024], F32, tag="pavB")
                for hh, pav in ((0, pavA), (1, pavB)):
                    h = 2 * hp + hh
                    for jt in range(2):
                        for half in range(2):
                            nc.tensor.matmul(
                                pav[:, half * 512:(half + 1) * 512],
                                vT_sb[:, jt, h * 128:(h + 1) * 128],
                                E[:, hh, jt, sq * 1024 + half * 512:
                                   sq * 1024 + (half + 1) * 512],
                                start=(jt == 0), stop=(jt == 1))
                zsb = zpool.tile([128, 1024], F32, tag="zsb")
                nc.scalar.activation(zsb[0:64, :], pavA[64:128, :], AF.Copy)
                nc.vector.tensor_copy(zsb[64:128, :], pavB[64:128, :])
                rec = zpool.tile([128, 1024], F32, tag="rec")
                nc.vector.reciprocal_approx_fast(rec[:], zsb[:])
                nc.vector.tensor_tensor(
                    out=outT_sb[0:64, hp, sq * 1024:(sq + 1) * 1024],
                    in0=pavA[0:64, :], in1=rec[0:64, :], op=OP.mult)
                if sq % 2 == 0:
                    nc.vector.tensor_tensor(
                        out=outT_sb[64:128, hp, sq * 1024:(sq + 1) * 1024],
                        in0=pavB[0:64, :], in1=rec[64:128, :], op=OP.mult)
                else:
                    nsb = zpool.tile([128, 1024], F32, tag="nsb")
                    nc.scalar.activation(nsb[64:128, :], pavB[0:64, :], AF.Copy)
                    nc.gpsimd.tensor_tensor(
                        out=outT_sb[64:128, hp, sq * 1024:(sq + 1) * 1024],
                        in0=nsb[64:128, :], in1=rec[64:128, :], op=OP.mult)
        if "outT" in taps:
            dbg = tc.alloc_tile_pool(name="dbgO", bufs=2)
            for c in range(4):
                for qq in range(4):
                    ev = dbg.tile([128, 1024], F32, tag="dbgO")
                    nc.vector.tensor_copy(ev[:], outT_sb[:, c, qq * 1024:(qq + 1) * 1024])
                    nc.sync.dma_start(
                        taps["outT"].ap()[c * 128:(c + 1) * 128,
                                          qq * 1024:(qq + 1) * 1024], ev[:])
            dbg.release()
        psAV.release(); psS.release(); zpool.release(); ep.release()
        q_pool.release()

        # ---------------- final projection ----------------
        psF = tc.alloc_tile_pool(name="psF", bufs=4, space="PSUM")
        yev = tc.alloc_tile_pool(name="yev", bufs=3)
        for oc in range(4):
            for sb2 in range(8):
                pf = psF.tile([128, 512], F32, tag="pf")
                for ic in range(4):
                    nc.tensor.matmul(pf[:], wo_sb[:, ic, oc * 128:(oc + 1) * 128],
                                     outT_sb[:, ic, sb2 * 512:(sb2 + 1) * 512],
                                     start=(ic == 0), stop=(ic == 3))
                ye = yev.tile([128, 512], F32, tag="ye")
                if sb2 % 2 == 0:
                    nc.scalar.activation(ye[:], pf[:], AF.Identity,
                                         bias=bout_sb[:, oc:oc + 1])
                else:
                    nc.vector.tensor_scalar(out=ye[:], in0=pf[:],
                                            scalar1=bout_sb[:, oc:oc + 1],
                                            scalar2=None, op0=OP.add)
                nc.sync.dma_start(
                    y_out.ap()[oc * 128:(oc + 1) * 128, sb2 * 512:(sb2 + 1) * 512],
                    ye[:])
        yev.release(); psF.release()
        outT_pool.release(); drp.release(); P0.release()

    with tile.TileContext(nc) as tc:
        _pipeline(tc)
    nc.compile()
    return nc


# ---------------------------------------------------------------------------
_NC_CACHE = {}


def _get_nc():
    if "nc" not in _NC_CACHE:
        _NC_CACHE["nc"] = build()
    return _NC_CACHE["nc"]


def kernel(x, w_q, w_off1, b_off1, w_off2, w_kv, w_out, b_out):
    from concourse.bass_utils import run_bass_kernel_spmd
    bf = ml_dtypes.bfloat16
    x = np.asarray(x, np.float32)
    b = x.shape[0]
    assert x.shape == (8, DIM, H, W), f"unexpected x shape {x.shape}"
    wd = prep_weights(w_q, w_off1, b_off1, w_off2, w_kv, w_out, b_out)
    in_maps = [{"x": np.ascontiguousarray(x[i].reshape(DIM, S)).astype(bf), **wd}
               for i in range(b)]
    nc = _get_nc()
    res = run_bass_kernel_spmd(nc, in_maps, core_ids=list(range(b)))
    out = np.stack([res.results[i]["y"].reshape(DIM, H, W) for i in range(b)])
    return out.astype(np.float32)
